# revision 1
# baseline (speedup 1.0000x reference)
"""Causal self-attention (B=2, T=4096, C=768, H=12) on 8 trn2 NeuronCores.

Sharding: data-parallel on batch (cores 0-3 -> batch 0, cores 4-7 -> batch 1),
tensor-parallel on heads (3 heads per core).  Each core computes qkv for its
3 heads, causal flash-style attention, and a partial output projection
(its heads' rows of w_proj); the host sums the 4 partials per batch.

All matmuls run in fp32r (TF32-like, 13-bit mantissa, full PE speed).
Attention is computed in a transposed layout (S^T tiles = K_tile^T x Q) so
softmax sums come from a ones-column appended to V, and no transposes are
needed in the inner loop.

Perf structure (v7): the kernel is ScalarE(exp)-latency-bound, so the
attention inner loop batches two 128x512 S-tiles into one [128,1024] exp
(halving ACT instruction count), interleaves heads 0/1's k-loops at pair
granularity to keep independent exps in flight, double-buffers both the
projection output staging and the qkv input staging, and keeps ScalarE
free of copies so attention exps are never queued behind them (measured
regression when x-rounding ran on ScalarE).  The final diagonal pair of
each query block runs at half width (its keys are invisible to queries
in columns < 256), and the Y^T staging buffer is double-buffered so each
query block's normalize tail overlaps the next block's accumulation.
Measured ~0.65-0.68 ms per iteration on HW (repeat-16 vs repeat-8 wall
differencing), down from ~0.88-0.94 ms for the unpipelined version.
"""

import sys

if '/opt/trn_rl_repo' not in sys.path:
    sys.path.insert(0, '/opt/trn_rl_repo')

import numpy as np

import concourse.bacc as bacc
import concourse.mybir as mybir
import concourse.tile as tile
from concourse.masks import make_identity

dt = mybir.dt
F32 = dt.float32
F32R = dt.float32r

N_EMBD = 768
N_HEADS = 12
HEAD_DIM = 64
B = 2
T_FULL = 4096
N_CORES = 8
HEADS_PER_CORE = N_HEADS // (N_CORES // B)  # 3

TOK_CHUNK = 256   # qkv phase token chunk
QSB = 512         # attention query superblock
KT = 128          # key tile (contraction for P@V)
CCHUNKS = N_EMBD // 128  # 6 contraction chunks


BUFS = {"pbig": 2, "py": 2, "psmall": 2, "ptp": 2, "ysb": 2, "yqn": 1, "xr": 2, "xs": 2}
C_MODE = "full"  # debug knob: full | noy | notail | nomask


def build_nc(T=T_FULL, repeat=1, phases=('B','B2','C','D')):
    """Build the per-core Bass program.  Same program runs SPMD on all 8
    cores; per-core data (x^T of its batch, its heads' weight slices) comes
    via the input map."""
    nc = bacc.Bacc(None, target_bir_lowering=False, debug=False)

    n_kt = T // KT
    n_qsb = T // QSB
    n_tok = T // 128

    XT = nc.dram_tensor("xt", [N_EMBD, T], F32, kind="ExternalInput")
    WQ01 = nc.dram_tensor("wq01", [N_EMBD, 128], F32, kind="ExternalInput")
    WK01 = nc.dram_tensor("wk01", [N_EMBD, 128], F32, kind="ExternalInput")
    WV01 = nc.dram_tensor("wv01", [N_EMBD, 128], F32, kind="ExternalInput")
    WQV2 = nc.dram_tensor("wqv2", [N_EMBD, 128], F32, kind="ExternalInput")
    WK2 = nc.dram_tensor("wk2", [N_EMBD, 64], F32, kind="ExternalInput")
    WP1 = nc.dram_tensor("wp1", [128, N_EMBD], F32, kind="ExternalInput")
    WP2 = nc.dram_tensor("wp2", [64, N_EMBD], F32, kind="ExternalInput")
    Y = nc.dram_tensor("y", [T, N_EMBD], F32, kind="ExternalOutput")

    xt_ap = XT.ap().rearrange("(c p) t -> p c t", p=128)

    with tile.TileContext(nc) as tc:
        with (
            tc.tile_pool(name="const", bufs=1) as const_pool,
            tc.tile_pool(name="wpool", bufs=1) as wpool,
            tc.tile_pool(name="wstage", bufs=1) as wstage,
            tc.tile_pool(name="qkvt", bufs=1) as qkvt,
            tc.tile_pool(name="vsb", bufs=1) as vsb_pool,
            tc.tile_pool(name="ynt", bufs=1) as ynt_pool,
            tc.tile_pool(name="xs", bufs=BUFS["xs"]) as xs_pool,
            tc.tile_pool(name="xr", bufs=BUFS["xr"]) as xr_pool,
            tc.tile_pool(name="ptp", bufs=BUFS["ptp"]) as pt_pool,
            tc.tile_pool(name="ysb", bufs=BUFS["ysb"]) as ysb_pool,
            tc.tile_pool(name="rp", bufs=4) as r_pool,
            tc.tile_pool(name="yout", bufs=2) as yout_pool,
            tc.tile_pool(name="yqn", bufs=BUFS["yqn"]) as yqn_pool,
            tc.tile_pool(name="pbig", bufs=BUFS["pbig"], space="PSUM") as pbig,
            tc.tile_pool(name="py", bufs=BUFS["py"], space="PSUM") as py_pool,
            tc.tile_pool(name="psmall", bufs=BUFS["psmall"], space="PSUM") as psmall,
        ):
            # ---- constants (built in fp32 scratch, rounded to fp32r) ----
            ident_f = const_pool.tile([128, 128], F32)
            make_identity(nc, ident_f)
            ident = const_pool.tile([128, 128], F32R)
            nc.vector.tensor_copy(out=ident, in_=ident_f)
            # causal mask master: M[i, c] = 1.0 iff c >= i + 384.
            # slice [384-d : 896-d] gives tile-mask for diag offset d.
            mask_f = wstage.tile([128, QSB + 384], F32, tag="wst")
            nc.gpsimd.memset(mask_f, 1.0)
            nc.gpsimd.affine_select(
                out=mask_f, in_=mask_f,
                compare_op=mybir.AluOpType.is_ge,
                fill=0.0, base=-384, channel_multiplier=-1,
                pattern=[[1, QSB + 384]],
            )
            mask = const_pool.tile([128, QSB + 384], F32R)
            nc.vector.tensor_copy(out=mask, in_=mask_f)

            # ---- weights: load + round to fp32r ----
            def load_w(src_ap, shape_r, tag):
                st = wstage.tile(shape_r, F32, tag="wst")
                nc.sync.dma_start(out=st, in_=src_ap)
                rt = wpool.tile(shape_r, F32R, tag=tag)
                nc.vector.tensor_copy(out=rt, in_=st)
                return rt

            wq01r = load_w(WQ01.ap().rearrange("(c p) m -> p c m", p=128), [128, CCHUNKS, 128], tag="wq01r")
            wk01r = load_w(WK01.ap().rearrange("(c p) m -> p c m", p=128), [128, CCHUNKS, 128], tag="wk01r")
            wv01r = load_w(WV01.ap().rearrange("(c p) m -> p c m", p=128), [128, CCHUNKS, 128], tag="wv01r")
            wqv2r = load_w(WQV2.ap().rearrange("(c p) m -> p c m", p=128), [128, CCHUNKS, 128], tag="wqv2r")
            wk2r = load_w(WK2.ap().rearrange("(c p) m -> p c m", p=128), [128, CCHUNKS, 64], tag="wk2r")
            wp1r = load_w(WP1.ap(), [128, N_EMBD], tag="wp1r")
            wp2r = load_w(WP2.ap(), [64, N_EMBD], tag="wp2r")

            # ---- persistent activations ----
            QT01 = qkvt.tile([128, T], F32R, tag="qt01")
            KT01 = qkvt.tile([128, T], F32R, tag="kt01")
            VT01 = qkvt.tile([128, T], F32R, tag="vt01")
            QV2 = qkvt.tile([128, T], F32R, tag="qv2")   # q_h2 rows 0:64, v_h2 rows 64:128
            KT2 = qkvt.tile([64, T], F32R, tag="kt2")
            Vsb = vsb_pool.tile([128, n_kt, HEADS_PER_CORE, 65], F32R)
            YnT01 = ynt_pool.tile([128, T], F32R, tag="ynt01")
            YnT2 = ynt_pool.tile([64, T], F32R, tag="ynt2")

            ones_f = const_pool.tile([128, n_kt * HEADS_PER_CORE], F32)
            nc.vector.memset(ones_f, 1.0)
            nc.vector.tensor_copy(
                out=Vsb[:, :, :, 64:65].rearrange("p a b c -> p (a b c)"),
                in_=ones_f)

            for _ in range(repeat):
                # ================= phase B: qkv projections ================
                # out tensors are [M, tok] with M = packed head-dim rows:
                #   QT01 = [q_h0; q_h1], KT01 = [k_h0; k_h1], VT01 = [v_h0; v_h1],
                #   QK2 = [q_h2; k_h2], VT2 = [v_h2]
                qkv_jobs = [
                    (wq01r, QT01, 128), (wk01r, KT01, 128), (wv01r, VT01, 128),
                    (wqv2r, QV2, 128), (wk2r, KT2, 64),
                ]
                for ch in range(T // TOK_CHUNK if 'B' in phases else 0):
                    sl = slice(ch * TOK_CHUNK, (ch + 1) * TOK_CHUNK)
                    xs = xs_pool.tile([128, CCHUNKS, TOK_CHUNK], F32)
                    nc.sync.dma_start(out=xs, in_=xt_ap[:, :, sl])
                    xr = xr_pool.tile([128, CCHUNKS, TOK_CHUNK], F32R)
                    nc.vector.tensor_copy(out=xr, in_=xs)
                    for wt, out_sb, m in qkv_jobs:
                        ps = pbig.tile([128, TOK_CHUNK], F32, tag="big")
                        for c in range(CCHUNKS):
                            nc.tensor.matmul(
                                ps[0:m, :], wt[:, c, 0:m], xr[:, c, :],
                                start=(c == 0), stop=(c == CCHUNKS - 1),
                            )
                        nc.vector.tensor_copy(out=out_sb[0:m, sl], in_=ps[0:m, :])

                # ========== phase B2: V^T -> V (keys-major) transposes =====
                for h in range(HEADS_PER_CORE if 'B2' in phases else 0):
                    for kt in range(n_kt):
                        ks = slice(kt * KT, (kt + 1) * KT)
                        if h == 0:
                            src, idn = VT01[0:64, ks], ident[0:64, 0:64]
                        elif h == 1:
                            src, idn = VT01[64:128, ks], ident[64:128, 64:128]
                        else:
                            src, idn = QV2[64:128, ks], ident[64:128, 64:128]
                        pv = psmall.tile([128, 64], F32R, tag="small")
                        nc.tensor.transpose(pv[:, 0:64], src, idn)
                        nc.vector.tensor_copy(out=Vsb[:, kt, h, 0:64], in_=pv[:, 0:64])

                # ================= phase C: attention ======================
                head_qk = [
                    (QT01[0:64, :], KT01[0:64, :]),
                    (QT01[64:128, :], KT01[64:128, :]),
                    (QV2[0:64, :], KT2[0:64, :]),
                ]
                def attend_kloop_gen(h, qs, qt_h, kt_h, nkt_q, yps):
                    qsl = slice(qs * QSB, (qs + 1) * QSB)
                    for kt2 in range(0, nkt_q, 2):
                        yield
                        # the final (diagonal) pair has delta = (256, 384):
                        # queries in columns [0, 256) see none of its keys, so
                        # compute it at half width (columns 256:512 only).
                        last = (kt2 == nkt_q - 2)
                        q0 = QSB // 2 if last else 0
                        wsl = slice(q0, QSB)
                        sps2 = pbig.tile([128, 2, QSB], F32, tag="big")
                        for j in range(2):
                            kt = kt2 + j
                            ksl = slice(kt * KT, (kt + 1) * KT)
                            nc.tensor.matmul(sps2[:, j, wsl], kt_h[:, ksl],
                                             qt_h[:, qs * QSB + q0:(qs + 1) * QSB],
                                             start=True, stop=True)
                        pt2 = pt_pool.tile([128, 2, QSB], F32R)
                        nc.scalar.activation(
                            out=pt2[:, :, wsl], in_=sps2[:, :, wsl],
                            func=mybir.ActivationFunctionType.Exp,
                            scale=float(HEAD_DIM) ** -0.5,
                        )
                        for j in range(2):
                            kt = kt2 + j
                            delta = kt * KT - qs * QSB
                            if delta >= -KT + 1 and C_MODE != "nomask":
                                nc.vector.tensor_mul(
                                    pt2[:, j, wsl], pt2[:, j, wsl],
                                    mask[:, 384 - delta + q0: 384 - delta + QSB])
                        if C_MODE == "noy":
                            continue
                        for j in range(2):
                            kt = kt2 + j
                            nc.tensor.matmul(yps[:, wsl], Vsb[:, kt, h, :],
                                             pt2[:, j, wsl],
                                             start=(kt == 0),
                                             stop=(kt == nkt_q - 1))

                def attend_pair(qs, hs):
                    """Interleave the k-loops of the heads in `hs` at pair
                    granularity so ScalarE always has an independent exp
                    ready (hides cross-engine latency)."""
                    nkt_q = (qs + 1) * (QSB // KT)
                    ypss = {}
                    for h in hs:
                        ypss[h] = py_pool.tile([65, QSB], F32, tag="y",
                                               name=f"yps{h}")
                    gens = {h: attend_kloop_gen(h, qs, *head_qk[h], nkt_q, ypss[h])
                            for h in hs}
                    live = dict(gens)
                    while live:
                        for h in list(live):
                            try:
                                next(live[h])
                            except StopIteration:
                                del live[h]
                    return ypss

                def finish_qsb(h, qs, yps):
                        if C_MODE in ("noy", "notail"):
                            return
                        ysb = ysb_pool.tile([65, QSB], F32)
                        nc.vector.tensor_copy(out=ysb, in_=yps)
                        # transpose + normalize 128-query chunks
                        for qt in range(QSB // 128):
                            csl = slice(qs * QSB + qt * 128, qs * QSB + (qt + 1) * 128)
                            pt1 = psmall.tile([128, 65], F32, tag="small")
                            nc.tensor.transpose(
                                pt1, ysb[:, qt * 128:(qt + 1) * 128], ident_f[0:65, 0:65])
                            rr = r_pool.tile([128, 1], F32)
                            nc.vector.reciprocal(rr, pt1[:, 64:65])
                            yqn = yqn_pool.tile([128, 64], F32R)
                            nc.vector.tensor_scalar_mul(yqn, pt1[:, 0:64], rr)
                            pt2 = psmall.tile([64, 128], F32R, tag="small")
                            nc.tensor.transpose(pt2, yqn, ident)
                            if h == 0:
                                dst = YnT01[0:64, csl]
                            elif h == 1:
                                dst = YnT01[64:128, csl]
                            else:
                                dst = YnT2[0:64, csl]
                            nc.vector.tensor_copy(out=dst, in_=pt2)

                def attend_qs_pair(h, qs_list):
                    """Interleave one head's k-loops across two query blocks
                    (used for head 2, which has no partner head)."""
                    gens, ypss = {}, {}
                    for qs in qs_list:
                        nkt_q = (qs + 1) * (QSB // KT)
                        ypss[qs] = py_pool.tile([65, QSB], F32, tag="y",
                                                name=f"yps2_{qs}")
                        gens[qs] = attend_kloop_gen(h, qs, *head_qk[h],
                                                    nkt_q, ypss[qs])
                    live = dict(gens)
                    while live:
                        for qs in list(live):
                            try:
                                next(live[qs])
                            except StopIteration:
                                del live[qs]
                    return ypss

                if 'C' in phases:
                    for qs in range(n_qsb):
                        for hs in ((0, 1), (2,)):
                            ypss = attend_pair(qs, hs)
                            for h in hs:
                                finish_qsb(h, qs, ypss[h])

                # ================= phase D: partial projection =============
                for tt in range(n_tok if 'D' in phases else 0):
                    tsl = slice(tt * 128, (tt + 1) * 128)
                    yo = yout_pool.tile([128, N_EMBD], F32)
                    for c0, ncols in ((0, 512), (512, 256)):
                        pp = pbig.tile([128, 512], F32, tag="big")
                        nc.tensor.matmul(pp[:, 0:ncols], YnT01[:, tsl],
                                         wp1r[:, c0:c0 + ncols], start=True, stop=False)
                        nc.tensor.matmul(pp[:, 0:ncols], YnT2[0:64, tsl],
                                         wp2r[0:64, c0:c0 + ncols], start=False, stop=True)
                        nc.vector.tensor_copy(out=yo[:, c0:c0 + ncols], in_=pp[:, 0:ncols])
                    nc.sync.dma_start(out=Y.ap()[tsl, :], in_=yo)

    nc.compile()
    return nc


def make_in_maps(x, w_qkv, w_proj, T=T_FULL):
    """Per-core input dicts from full inputs (numpy)."""
    x = np.asarray(x, dtype=np.float32)
    w_qkv = np.asarray(w_qkv, dtype=np.float32)
    w_proj = np.asarray(w_proj, dtype=np.float32)
    cores_per_batch = N_CORES // B
    xt_b = [np.ascontiguousarray(x[b].T) for b in range(B)]  # [768, T]
    in_maps = []
    for core in range(N_CORES):
        b = core // cores_per_batch
        h0 = (core % cores_per_batch) * HEADS_PER_CORE
        h1, h2 = h0 + 1, h0 + 2
        col = lambda kind, h: w_qkv[:, kind * N_EMBD + h * HEAD_DIM:
                                    kind * N_EMBD + (h + 1) * HEAD_DIM]
        in_maps.append({
            "xt": xt_b[b],
            "wq01": np.ascontiguousarray(np.concatenate([col(0, h0), col(0, h1)], axis=1)),
            "wk01": np.ascontiguousarray(np.concatenate([col(1, h0), col(1, h1)], axis=1)),
            "wv01": np.ascontiguousarray(np.concatenate([col(2, h0), col(2, h1)], axis=1)),
            "wqv2": np.ascontiguousarray(np.concatenate([col(0, h2), col(2, h2)], axis=1)),
            "wk2": np.ascontiguousarray(col(1, h2)),
            "wp1": np.ascontiguousarray(w_proj[h0 * HEAD_DIM:(h1 + 1) * HEAD_DIM, :]),
            "wp2": np.ascontiguousarray(w_proj[h2 * HEAD_DIM:(h2 + 1) * HEAD_DIM, :]),
        })
    return in_maps


def gather_output(results, T=T_FULL):
    cores_per_batch = N_CORES // B
    out = np.empty((B, T, N_EMBD), dtype=np.float32)
    for b in range(B):
        parts = [results[b * cores_per_batch + j]["y"] for j in range(cores_per_batch)]
        out[b] = parts[0] + parts[1] + parts[2] + parts[3]
    return out


_CACHE = {}


def _get_nc(T=T_FULL, repeat=1):
    key = (T, repeat)
    if key not in _CACHE:
        _CACHE[key] = build_nc(T, repeat)
    return _CACHE[key]


def kernel(x, w_qkv, w_proj):
    import time as _time
    from concourse.bass_utils import run_bass_kernel_spmd
    T = x.shape[1]
    nc = _get_nc(T)
    in_maps = make_in_maps(x, w_qkv, w_proj, T)
    last_err = None
    for attempt in range(3):
        try:
            res = run_bass_kernel_spmd(nc, in_maps, list(range(N_CORES)))
            return gather_output(res.results, T)
        except Exception as e:  # transient device wedge: retry after a pause
            last_err = e
            _time.sleep(20 * (attempt + 1))
    raise last_err



# revision 3
# speedup vs baseline: 1.0506x; 1.0506x over previous
"""Causal self-attention (B=2, T=4096, C=768, H=12) on 8 trn2 NeuronCores. v11.

Sharding: data-parallel on batch (cores 0-3 -> batch 0, cores 4-7 -> batch 1),
tensor-parallel on heads (3 heads per core); host sums 4 projection partials
per batch.

v11 vs v8: phase-attribution on HW showed the normalize tails (+101us) and
projection tiles (+117us) were serializing on PSUM slots instead of hiding
under attention. Changes:
- exps un-paired ([128,512] tiles): S psum tiles are 1 bank, freeing banks
  for a dedicated 2-buf projection pool (pmix) so D pipelines.
- V computed token-major directly from x (stationary-x matmul, 192 v-cols
  moving) and evicted straight into the keys-major Vsb layout: the whole
  V-transpose phase (96 PE transposes + psmall lane) is gone.
- q_h2|k_h2 merged into one qkv job (one 128-col weight matrix).
- finish tails: heads 0+1 share one merged transpose chain per 128 queries.
"""

import sys

if '/opt/trn_rl_repo' not in sys.path:
    sys.path.insert(0, '/opt/trn_rl_repo')

import numpy as np

import concourse.bacc as bacc
import concourse.mybir as mybir
import concourse.tile as tile
from concourse.masks import make_identity

dt = mybir.dt
F32 = dt.float32
BF16 = dt.bfloat16

N_EMBD = 768
N_HEADS = 12
HEAD_DIM = 64
B = 2
T_FULL = 4096
N_CORES = 8
HEADS_PER_CORE = N_HEADS // (N_CORES // B)  # 3

QSB = 512         # attention query superblock
KT = 128          # key tile
CCHUNKS = N_EMBD // 128  # 6 contraction chunks
XCHUNK = 1024     # x DMA chunk (tokens); matmul sub-chunk is 512


def build_nc(T=T_FULL, repeat=1, phases=('B', 'C', 'F', 'D')):
    nc = bacc.Bacc(None, target_bir_lowering=False, debug=False)

    n_qsb = T // QSB
    n_kt = T // KT
    n_chunk = T // XCHUNK
    kt_per_chunk = XCHUNK // KT

    XT = nc.dram_tensor("xt", [N_EMBD, T], BF16, kind="ExternalInput")
    WQ01 = nc.dram_tensor("wq01", [N_EMBD, 128], BF16, kind="ExternalInput")
    WK01 = nc.dram_tensor("wk01", [N_EMBD, 128], BF16, kind="ExternalInput")
    WQ2 = nc.dram_tensor("wq2", [N_EMBD, 64], BF16, kind="ExternalInput")
    WK2 = nc.dram_tensor("wk2", [N_EMBD, 64], BF16, kind="ExternalInput")
    WV3 = nc.dram_tensor("wv3", [N_EMBD, 192], BF16, kind="ExternalInput")
    WP1 = nc.dram_tensor("wp1", [128, N_EMBD], BF16, kind="ExternalInput")
    WP2 = nc.dram_tensor("wp2", [64, N_EMBD], BF16, kind="ExternalInput")
    Y = nc.dram_tensor("y", [T, N_EMBD], F32, kind="ExternalOutput")

    xt_ap = XT.ap().rearrange("(c p) t -> p c t", p=128)

    with tile.TileContext(nc) as tc:
        with (
            tc.tile_pool(name="const", bufs=1) as const_pool,
            tc.tile_pool(name="wpool", bufs=1) as wpool,
            tc.tile_pool(name="qkvt", bufs=1) as qkvt,
            tc.tile_pool(name="vsb", bufs=1) as vsb_pool,
            tc.tile_pool(name="ynt", bufs=1) as ynt_pool,
            tc.tile_pool(name="xs", bufs=2) as xs_pool,
            tc.tile_pool(name="ptp", bufs=6) as ptp_pool,
            tc.tile_pool(name="ysb", bufs=3) as ysb_pool,
            tc.tile_pool(name="rp", bufs=4) as r_pool,
            tc.tile_pool(name="yqn", bufs=3) as yqn_pool,
            tc.tile_pool(name="yout", bufs=3) as yout_pool,
            tc.tile_pool(name="sp", bufs=2, space="PSUM") as sp_pool,
            tc.tile_pool(name="pacc", bufs=3, space="PSUM") as pacc_pool,
            tc.tile_pool(name="pmix", bufs=2, space="PSUM") as pmix_pool,
            tc.tile_pool(name="psmall", bufs=1, space="PSUM") as psmall_pool,
        ):
            # ---- constants ----
            ident_f = const_pool.tile([128, 128], F32)
            make_identity(nc, ident_f)
            ident = const_pool.tile([128, 128], BF16)
            nc.vector.tensor_copy(out=ident, in_=ident_f)
            # causal mask master: M[i, c] = 1.0 iff c >= i + 384.
            mask_f = const_pool.tile([128, QSB + 384], F32)
            nc.gpsimd.memset(mask_f, 1.0)
            nc.gpsimd.affine_select(
                out=mask_f, in_=mask_f,
                compare_op=mybir.AluOpType.is_ge,
                fill=0.0, base=-384, channel_multiplier=-1,
                pattern=[[1, QSB + 384]],
            )
            mask = const_pool.tile([128, QSB + 384], BF16)
            nc.vector.tensor_copy(out=mask, in_=mask_f)

            # ---- weights: direct bf16 DMA ----
            def load_w(src_ap, shape, tag):
                t = wpool.tile(shape, BF16, tag=tag)
                nc.sync.dma_start(out=t, in_=src_ap)
                return t

            wq01 = load_w(WQ01.ap().rearrange("(c p) m -> p c m", p=128),
                          [128, CCHUNKS, 128], "wq01")
            wk01 = load_w(WK01.ap().rearrange("(c p) m -> p c m", p=128),
                          [128, CCHUNKS, 128], "wk01")
            wq2 = load_w(WQ2.ap().rearrange("(c p) m -> p c m", p=128),
                         [128, CCHUNKS, 64], "wq2")
            wk2 = load_w(WK2.ap().rearrange("(c p) m -> p c m", p=128),
                         [128, CCHUNKS, 64], "wk2")
            wv3 = load_w(WV3.ap().rearrange("(c p) m -> p c m", p=128),
                         [128, CCHUNKS, 192], "wv3")
            wp1 = load_w(WP1.ap(), [128, N_EMBD], "wp1")
            wp2 = load_w(WP2.ap(), [64, N_EMBD], "wp2")

            # ---- persistent activations ----
            QT01 = qkvt.tile([128, T], BF16, tag="qt01")
            KT01 = qkvt.tile([128, T], BF16, tag="kt01")
            Q2 = qkvt.tile([64, T], BF16, tag="q2")
            K2 = qkvt.tile([64, T], BF16, tag="k2")
            Vsb = vsb_pool.tile([128, n_kt, HEADS_PER_CORE, 65], BF16)
            YnT01 = ynt_pool.tile([128, T], BF16, tag="ynt01")
            YnT2 = ynt_pool.tile([64, T], BF16, tag="ynt2")

            ones_f = const_pool.tile([128, n_kt * HEADS_PER_CORE], F32)
            nc.vector.memset(ones_f, 1.0)
            nc.vector.tensor_copy(
                out=Vsb[:, :, :, 64:65].rearrange("p a b c -> p (a b c)"),
                in_=ones_f)

            qk_jobs = [
                (wq01, QT01, 128), (wk01, KT01, 128),
                (wq2, Q2, 64), (wk2, K2, 64),
            ]
            head_qk = [
                (QT01[0:64, :], KT01[0:64, :]),
                (QT01[64:128, :], KT01[64:128, :]),
                (Q2[0:64, :], K2[0:64, :]),
            ]

            for _ in range(repeat):
                # ---- generators ----
                def gen_B(c):
                    """qkv for token chunk c: q/k dim-major jobs + token-major
                    V accumulated straight into the keys-major Vsb layout."""
                    csl = slice(c * XCHUNK, (c + 1) * XCHUNK)
                    xs = xs_pool.tile([128, CCHUNKS, XCHUNK], BF16)
                    nc.sync.dma_start(out=xs, in_=xt_ap[:, :, csl])
                    yield
                    for half in range(XCHUNK // 512):
                        t0 = c * XCHUNK + half * 512
                        for wt, out_sb, m in qk_jobs:
                            psb = pmix_pool.tile([128, 512], F32, tag="mix")
                            for cc in range(CCHUNKS):
                                nc.tensor.matmul(
                                    psb[0:m, :], wt[:, cc, 0:m],
                                    xs[:, cc, half * 512:(half + 1) * 512],
                                    start=(cc == 0), stop=(cc == CCHUNKS - 1))
                            nc.vector.tensor_copy(
                                out=out_sb[0:m, t0:t0 + 512], in_=psb[0:m, :])
                            yield
                    for tt in range(kt_per_chunk):
                        kt = c * kt_per_chunk + tt
                        tsl = slice(tt * KT, (tt + 1) * KT)
                        pv = pmix_pool.tile([128, 192], F32, tag="mix")
                        for cc in range(CCHUNKS):
                            nc.tensor.matmul(
                                pv, xs[:, cc, tsl], wv3[:, cc, :],
                                start=(cc == 0), stop=(cc == CCHUNKS - 1))
                        nc.vector.tensor_copy(
                            out=Vsb[:, kt, :, 0:64],
                            in_=pv.rearrange("p (h d) -> p h d", h=3))
                        yield

                def gen_C(qs, h, yps):
                    """Attention k-loop (single-tile granularity) for
                    (query superblock qs, head h)."""
                    qt_h, kt_h = head_qk[h]
                    nkt_q = (qs + 1) * (QSB // KT)
                    for kt in range(nkt_q):
                        yield
                        q0 = QSB // 2 if kt >= nkt_q - 2 else 0
                        wsl = slice(q0, QSB)
                        sps = sp_pool.tile([128, QSB], F32, tag="s")
                        nc.tensor.matmul(
                            sps[:, wsl], kt_h[:, kt * KT:(kt + 1) * KT],
                            qt_h[:, qs * QSB + q0:(qs + 1) * QSB],
                            start=True, stop=True)
                        pt2 = ptp_pool.tile([128, QSB], BF16)
                        nc.scalar.activation(
                            out=pt2[:, wsl], in_=sps[:, wsl],
                            func=mybir.ActivationFunctionType.Exp,
                            scale=float(HEAD_DIM) ** -0.5)
                        delta = kt * KT - qs * QSB
                        if delta >= -KT + 1:
                            nc.vector.tensor_mul(
                                pt2[:, wsl], pt2[:, wsl],
                                mask[:, 384 - delta + q0:384 - delta + QSB])
                        nc.tensor.matmul(yps[:, wsl], Vsb[:, kt, h, :],
                                         pt2[:, wsl],
                                         start=(kt == 0),
                                         stop=(kt == nkt_q - 1))

                def gen_F01(qs, yps0, yps1):
                    """Merged normalize tail for heads 0+1 of superblock qs."""
                    ysb0 = ysb_pool.tile([65, QSB], BF16, tag="ysb", name="y0")
                    nc.vector.tensor_copy(out=ysb0, in_=yps0)
                    ysb1 = ysb_pool.tile([65, QSB], BF16, tag="ysb", name="y1")
                    nc.vector.tensor_copy(out=ysb1, in_=yps1)
                    yield
                    for qt in range(QSB // 128):
                        cs = slice(qt * 128, (qt + 1) * 128)
                        csl = slice(qs * QSB + qt * 128, qs * QSB + (qt + 1) * 128)
                        ptb = psmall_pool.tile([128, 132], BF16, tag="small")
                        nc.tensor.transpose(ptb[:, 0:65], ysb0[:, cs],
                                            ident[0:65, 0:65])
                        nc.tensor.transpose(ptb[:, 66:131], ysb1[:, cs],
                                            ident[0:65, 0:65])
                        rr = r_pool.tile([128, 2], F32)
                        nc.vector.reciprocal(rr[:, 0:1], ptb[:, 64:65])
                        nc.vector.reciprocal(rr[:, 1:2], ptb[:, 130:131])
                        yqn = yqn_pool.tile([128, 128], BF16)
                        nc.vector.tensor_scalar_mul(yqn[:, 0:64], ptb[:, 0:64],
                                                    rr[:, 0:1])
                        nc.vector.tensor_scalar_mul(yqn[:, 64:128],
                                                    ptb[:, 66:130], rr[:, 1:2])
                        ptc = psmall_pool.tile([128, 128], BF16, tag="small")
                        nc.tensor.transpose(ptc, yqn, ident)
                        nc.vector.tensor_copy(out=YnT01[:, csl], in_=ptc)
                        yield

                def gen_F2(qs, yps):
                    """Normalize tail for head 2."""
                    ysb = ysb_pool.tile([65, QSB], BF16, tag="ysb", name="y2")
                    nc.vector.tensor_copy(out=ysb, in_=yps)
                    yield
                    for qt in range(QSB // 128):
                        cs = slice(qt * 128, (qt + 1) * 128)
                        csl = slice(qs * QSB + qt * 128, qs * QSB + (qt + 1) * 128)
                        ptb = psmall_pool.tile([128, 65], BF16, tag="small")
                        nc.tensor.transpose(ptb, ysb[:, cs], ident[0:65, 0:65])
                        rr = r_pool.tile([128, 2], F32)
                        nc.vector.reciprocal(rr[:, 0:1], ptb[:, 64:65])
                        yqn = yqn_pool.tile([128, 128], BF16)
                        nc.vector.tensor_scalar_mul(yqn[:, 0:64], ptb[:, 0:64],
                                                    rr[:, 0:1])
                        ptc = psmall_pool.tile([64, 128], BF16, tag="small")
                        nc.tensor.transpose(ptc, yqn[:, 0:64], ident)
                        nc.vector.tensor_copy(out=YnT2[0:64, csl], in_=ptc)
                        yield

                def gen_D(tts):
                    """Partial output projection for 128-token tiles tts."""
                    for tt in tts:
                        tsl = slice(tt * 128, (tt + 1) * 128)
                        yo = yout_pool.tile([128, N_EMBD], F32)
                        for c0, ncols in ((0, 512), (512, 256)):
                            pp = pmix_pool.tile([128, 512], F32, tag="mix")
                            nc.tensor.matmul(pp[:, 0:ncols], YnT01[:, tsl],
                                             wp1[:, c0:c0 + ncols],
                                             start=True, stop=False)
                            nc.tensor.matmul(pp[:, 0:ncols], YnT2[0:64, tsl],
                                             wp2[0:64, c0:c0 + ncols],
                                             start=False, stop=True)
                            nc.vector.tensor_copy(out=yo[:, c0:c0 + ncols],
                                                  in_=pp[:, 0:ncols])
                            yield
                        nc.sync.dma_start(out=Y.ap()[tsl, :], in_=yo)

                def chain(*gens):
                    for g in gens:
                        yield from g

                def weave(gens):
                    live = list(gens)
                    while live:
                        for g in list(live):
                            try:
                                next(g)
                            except StopIteration:
                                live.remove(g)

                # ---- pipelined schedule ----
                # F(qs) and D(qs-1) are deferred into C(qs+1)'s weave so the
                # psmall-serialized chains hide under the next superblock's
                # attention instead of stalling the PE queue at the boundary.
                if 'B' in phases:
                    weave([gen_B(0)])
                pend_f = []
                for qs in range(n_qsb):
                    gens = []
                    if 'C' in phases:
                        ypss = {h: pacc_pool.tile([65, QSB], F32, tag="y",
                                                  name=f"yps{h}")
                                for h in range(HEADS_PER_CORE)}
                        for h in range(HEADS_PER_CORE):
                            gens.append(gen_C(qs, h, ypss[h]))
                    filler = list(pend_f)
                    pend_f = []
                    c = qs // 2 + 1
                    if 'B' in phases and qs % 2 == 0 and c < n_chunk:
                        filler.append(gen_B(c))
                    if 'D' in phases and qs >= 2:
                        filler.append(gen_D(range((qs - 2) * 4, (qs - 1) * 4)))
                    if filler:
                        gens.append(chain(*filler))
                    weave(gens)
                    if 'C' in phases and 'F' in phases:
                        pend_f = [gen_F01(qs, ypss[0], ypss[1]),
                                  gen_F2(qs, ypss[2])]
                # epilogue: drain deferred finishes + last projection tiles
                weave(pend_f)
                if 'D' in phases:
                    weave([gen_D(range((n_qsb - 2) * 4, n_qsb * 4))])

    nc.compile()
    return nc


def make_in_maps(x, w_qkv, w_proj, T=T_FULL):
    """Per-core input dicts from full inputs (numpy), converted to bf16."""
    import ml_dtypes
    BF = ml_dtypes.bfloat16
    x = np.asarray(x, dtype=np.float32)
    w_qkv = np.asarray(w_qkv, dtype=np.float32)
    w_proj = np.asarray(w_proj, dtype=np.float32)
    cores_per_batch = N_CORES // B
    xt_b = [np.ascontiguousarray(x[b].T).astype(BF) for b in range(B)]
    in_maps = []
    for core in range(N_CORES):
        b = core // cores_per_batch
        h0 = (core % cores_per_batch) * HEADS_PER_CORE
        h1, h2 = h0 + 1, h0 + 2
        col = lambda kind, h: w_qkv[:, kind * N_EMBD + h * HEAD_DIM:
                                    kind * N_EMBD + (h + 1) * HEAD_DIM]
        in_maps.append({
            "xt": xt_b[b],
            "wq01": np.ascontiguousarray(
                np.concatenate([col(0, h0), col(0, h1)], axis=1)).astype(BF),
            "wk01": np.ascontiguousarray(
                np.concatenate([col(1, h0), col(1, h1)], axis=1)).astype(BF),
            "wq2": np.ascontiguousarray(col(0, h2)).astype(BF),
            "wk2": np.ascontiguousarray(col(1, h2)).astype(BF),
            "wv3": np.ascontiguousarray(
                np.concatenate([col(2, h0), col(2, h1), col(2, h2)],
                               axis=1)).astype(BF),
            "wp1": np.ascontiguousarray(
                w_proj[h0 * HEAD_DIM:(h1 + 1) * HEAD_DIM, :]).astype(BF),
            "wp2": np.ascontiguousarray(
                w_proj[h2 * HEAD_DIM:(h2 + 1) * HEAD_DIM, :]).astype(BF),
        })
    return in_maps


def gather_output(results, T=T_FULL):
    cores_per_batch = N_CORES // B
    out = np.empty((B, T, N_EMBD), dtype=np.float32)
    for b in range(B):
        parts = [results[b * cores_per_batch + j]["y"]
                 for j in range(cores_per_batch)]
        out[b] = parts[0] + parts[1] + parts[2] + parts[3]
    return out


_CACHE = {}


def _get_nc(T=T_FULL, repeat=1):
    key = (T, repeat)
    if key not in _CACHE:
        _CACHE[key] = build_nc(T, repeat)
    return _CACHE[key]


def kernel(x, w_qkv, w_proj):
    import time as _time
    from concourse.bass_utils import run_bass_kernel_spmd
    T = x.shape[1]
    nc = _get_nc(T)
    in_maps = make_in_maps(x, w_qkv, w_proj, T)
    last_err = None
    for attempt in range(3):
        try:
            res = run_bass_kernel_spmd(nc, in_maps, list(range(N_CORES)))
            return gather_output(res.results, T)
        except Exception as e:  # transient device wedge: retry after a pause
            last_err = e
            _time.sleep(20 * (attempt + 1))
    raise last_err
